# revision 16
# baseline (speedup 1.0000x reference)
"""Trainium2 Bass kernel for CustomLSTMForecast.

B=512, T=256, I=256, H=512. Data-parallel: batch sharded 8 ways (64
rows/core), LSTM + fc weights replicated.

Per-core design (batch m = 64), fp8-DoubleRow recurrence:
  gates(t) = [h(t-1); x_t] @ (8*W_ext) + 8*bias   accumulated into two
  two-bank PSUM tiles (every matmul writes one bank at partition 0, as
  DoubleRow requires):
     PA[64,1024]: cols 0:512 = f-gate, 512:1024 = i-gate
     PB[64,1024]: cols 0:512 = c-hat,  512:1024 = o-gate
  The h-part runs in fp8e4 DoubleRow mode (K=256 per matmul, 2 k-pairs
  x 4 gates = 8 matmuls/step at half cost); h stored as fp8 loses only
  ~1e-2 final rel err (x-part in fp8 was 4e-2, so x stays bf16: 2
  k-chunks x 4 gates). Weights are pre-scaled by 8 on the host (keeps
  fp8 weights in a good range); the inverse 1/8 rides the ACT
  activations' free input scale. Bias is one Ki=1 fp8 DoubleRow matmul
  per gate (ones stationary, bias row moving).

  Elementwise (all partition-base 0): sigmoid(f,i) in one ACT op ->
  SBUF bf16; tanh(chat), sigmoid(o), tanh(c) -> SBUF bf16. The c/h tail
  is split into two hidden halves so each half's transposes, fp8 copy
  and next-step h-matmuls (A-bank gates first) overlap the other
  half's elementwise, shortening the recurrence critical path:
  u1 = c*sf (f32), u2 = tcb*si (bf16 2x), c = u1+u2 (f32),
  h = so*tanh(c) (bf16), h -> 2 PE transposes -> one PSUM->SBUF copy
  = the fp8 [128,2,64] stationary for that k-pair of step t+1.

  x/bias matmuls for t+1 are emitted before the elementwise of t so
  the PE stays busy during the elementwise tail; x is DMA'd in blocks
  of 8 steps, triple buffered.
"""
from contextlib import ExitStack

import numpy as np

import concourse.bass as bass
import concourse.tile as tile
from concourse import bacc, mybir
from concourse.bass_utils import run_bass_kernel_spmd

F32 = mybir.dt.float32
BF16 = mybir.dt.bfloat16
FP8 = mybir.dt.float8e4
AF = mybir.ActivationFunctionType
DR = mybir.MatmulPerfMode.DoubleRow

B, T, I, H = 512, 256, 256, 512
NCORES = 8
BC = B // NCORES          # 64 batch rows per core
KP = 3                    # fp8 k-pairs: 2 hidden (512) + 1 input (256)
WS = 8.0                  # host-side weight scale, undone by ACT scale
XB = 8                    # x DMA batch (steps per transfer)

# gate order in the NG axis of the W tile: f, i, chat, o
# (W_w row-chunk order is f, i, o, chat)
G_SRC = (0, 1, 3, 2)

_CACHE = {}


def _build(nsteps=T):
    if nsteps in _CACHE:
        return _CACHE[nsteps]
    nc = bacc.Bacc("TRN2", target_bir_lowering=False, debug=False,
                   num_devices=NCORES)
    nxb = (nsteps + XB - 1) // XB
    d_x = nc.dram_tensor("xb", [nxb, 128, XB, 2, BC], BF16,
                         kind="ExternalInput").ap()
    d_w = nc.dram_tensor("W8", [128, 2, 2, 4, 512], FP8,
                         kind="ExternalInput").ap()
    d_wx = nc.dram_tensor("Wx", [128, 2, 4, 512], BF16,
                          kind="ExternalInput").ap()
    d_br = nc.dram_tensor("br8", [1, 2, 4, 512], FP8,
                          kind="ExternalInput").ap()
    d_eye = nc.dram_tensor("eye", [BC, BC], BF16, kind="ExternalInput").ap()
    d_eyef = nc.dram_tensor("eyef", [BC, BC], F32,
                            kind="ExternalInput").ap()
    d_fcw = nc.dram_tensor("fcw", [BC, H], F32, kind="ExternalInput").ap()
    d_fcb = nc.dram_tensor("fcb", [BC, 1], F32, kind="ExternalInput").ap()
    d_out = nc.dram_tensor("out", [BC, 1], F32, kind="ExternalOutput").ap()

    with tile.TileContext(nc) as tc, ExitStack() as ctx:
        _body(tc, ctx, nsteps, d_x, d_w, d_wx, d_br, d_eye, d_eyef,
              d_fcw, d_fcb, d_out)
    nc.compile()
    _CACHE[nsteps] = nc
    return nc


def _body(tc, ctx, nsteps, d_x, d_w, d_wx, d_br, d_eye, d_eyef, d_fcw,
          d_fcb, d_out):
    nc = tc.nc
    const = ctx.enter_context(tc.tile_pool(name="const", bufs=1))
    xpool = ctx.enter_context(tc.tile_pool(name="x", bufs=3))
    gact = ctx.enter_context(tc.tile_pool(name="gact", bufs=2))
    state = ctx.enter_context(tc.tile_pool(name="state", bufs=2))
    psA = ctx.enter_context(tc.tile_pool(name="psA", bufs=2, space="PSUM"))
    psB = ctx.enter_context(tc.tile_pool(name="psB", bufs=1, space="PSUM"))
    psT = ctx.enter_context(tc.tile_pool(name="psT", bufs=1, space="PSUM"))

    sW = const.tile([128, 2, 2, 4, 512], FP8)
    nc.sync.dma_start(out=sW[:], in_=d_w)
    sWx = const.tile([128, 2, 4, 512], BF16)
    nc.sync.dma_start(out=sWx[:], in_=d_wx)
    s_ones = const.tile([1, 2, BC], FP8)
    nc.vector.memset(s_ones[:, 0, :], 1.0)
    nc.vector.memset(s_ones[:, 1, :], 0.0)
    s_br = const.tile([1, 2, 4, 512], FP8)
    nc.sync.dma_start(out=s_br[:], in_=d_br)
    s_eye = const.tile([BC, BC], BF16)
    nc.sync.dma_start(out=s_eye[:], in_=d_eye)
    s_eyef = const.tile([BC, BC], F32)
    nc.sync.dma_start(out=s_eyef[:], in_=d_eyef)
    s_fcw = const.tile([BC, H], F32)
    nc.sync.dma_start(out=s_fcw[:], in_=d_fcw)
    s_fcb = const.tile([BC, 1], F32)
    nc.sync.dma_start(out=s_fcb[:], in_=d_fcb)

    c_prev = [state.tile([BC, 256], F32, tag=f"c{i}", name=f"c{i}")
              for i in range(2)]
    nc.vector.memset(c_prev[0][:], 0.0)
    nc.vector.memset(c_prev[1][:], 0.0)

    xtiles = [None] * ((nsteps + XB - 1) // XB)

    def get_x(t):
        blk = t // XB
        if xtiles[blk] is None:
            xt = xpool.tile([128, XB, 2, BC], BF16, tag="xs")
            nc.sync.dma_start(out=xt[:], in_=d_x[blk])
            xtiles[blk] = xt
        return xtiles[blk][:, t % XB]

    def bank(pA, pB, g):
        ps = pA if g < 2 else pB
        return ps[:, 512 * (g % 2):512 * (g % 2) + 512]

    def emit_x_and_bias(t, pA, pB):
        """x-part (bf16) + bias (fp8 Ki=1 DoubleRow) matmuls for step t."""
        xs = get_x(t)                       # [128, 2, BC] bf16
        for g in range(4):
            out = bank(pA, pB, g)
            for j in range(2):
                nc.tensor.matmul(out, xs[:, j, :], sWx[:, j, g, :],
                                 start=(j == 0), stop=False)
            nc.tensor.matmul(out, s_ones[:], s_br[:, :, g, :],
                             start=False, stop=(t == 0), perf_mode=DR)

    def emit_hpart(pA, pB, hq, kp):
        """h-part fp8 DoubleRow matmuls for one k-pair (A gates first)."""
        for g in range(4):
            nc.tensor.matmul(bank(pA, pB, g), hq[:],
                             sW[:, kp, :, g, :],
                             start=False, stop=(kp == 1), perf_mode=DR)

    # prologue: step 0 gates have no h contribution
    pA = psA.tile([BC, 1024], F32, tag="A")
    pB = psB.tile([BC, 1024], F32, tag="B")
    emit_x_and_bias(0, pA, pB)

    h_half = None
    for t in range(nsteps):
        last = t == nsteps - 1
        if not last:
            pA_n = psA.tile([BC, 1024], F32, tag="A")
            pB_n = psB.tile([BC, 1024], F32, tag="B")
            emit_x_and_bias(t + 1, pA_n, pB_n)

        # gate activations for step t (PA = [f|i], PB = [chat|o])
        sfi = gact.tile([BC, 1024], BF16, tag="sfi")
        nc.scalar.activation(sfi[:], pA[:], AF.Sigmoid, scale=1.0 / WS)
        tcb = gact.tile([BC, 512], BF16, tag="tcb")
        nc.scalar.activation(tcb[:], pB[:, 0:512], AF.Tanh, scale=1.0 / WS)
        sigo = gact.tile([BC, 512], BF16, tag="sigo")
        nc.scalar.activation(sigo[:], pB[:, 512:1024], AF.Sigmoid,
                             scale=1.0 / WS)
        # c update, split into two hidden halves
        c_new = [state.tile([BC, 256], F32, tag=f"c{i}", name=f"c{i}")
                 for i in range(2)]
        u1 = [gact.tile([BC, 256], F32, tag=f"u1{i}", name=f"u1{i}")
              for i in range(2)]
        u2 = [gact.tile([BC, 256], BF16, tag=f"u2{i}", name=f"u2{i}")
              for i in range(2)]
        for i in range(2):
            s = 256 * i
            nc.vector.tensor_mul(u1[i][:], c_prev[i][:], sfi[:, s:s + 256])
            nc.vector.tensor_mul(u2[i][:], tcb[:, s:s + 256],
                                 sfi[:, 512 + s:512 + s + 256])
        for i in range(2):
            nc.vector.tensor_add(c_new[i][:], u1[i][:], u2[i][:])
        c_prev = c_new

        h_half = []
        for i in range(2):
            s = 256 * i
            tch = gact.tile([BC, 256], BF16, tag=f"tch{i}", name=f"tch{i}")
            nc.scalar.activation(tch[:], c_new[i][:], AF.Tanh)
            hh = state.tile([BC, 256], BF16, tag=f"h{i}", name=f"h{i}")
            nc.vector.tensor_mul(hh[:], sigo[:, s:s + 256], tch[:])
            h_half.append(hh)
            if not last:
                pT = psT.tile([128, 2 * BC], BF16, tag=f"hTp{i}",
                              name=f"hTp{i}")
                for j in range(2):
                    nc.tensor.transpose(pT[:, 64 * j:64 * j + 64],
                                        hh[:, 128 * j:128 * j + 128],
                                        s_eye[:])
                hq = state.tile([128, 2, BC], FP8, tag=f"hq{i}",
                                name=f"hq{i}")
                nc.vector.tensor_copy(hq[:], pT[:])
                emit_hpart(pA_n, pB_n, hq, i)
        if not last:
            pA, pB = pA_n, pB_n

    # fc head: out = h @ fc_w.T + fc_b
    m = gact.tile([BC, H], F32, tag="fcm")
    for i in range(2):
        nc.vector.tensor_mul(m[:, 256 * i:256 * i + 256], h_half[i][:],
                             s_fcw[:, 256 * i:256 * i + 256])
    r = gact.tile([BC, 1], F32, tag="fcr")
    nc.vector.tensor_reduce(r[:], m[:], axis=mybir.AxisListType.X,
                            op=mybir.AluOpType.add)
    ro = gact.tile([BC, 1], F32, tag="fco")
    nc.vector.tensor_add(ro[:], r[:], s_fcb[:])
    nc.sync.dma_start(out=d_out, in_=ro[:])


def _prep_shared(W_w, W_b, fc_w, fc_b):
    """Host-side weight relayout (core-independent)."""
    import ml_dtypes
    bf = ml_dtypes.bfloat16
    f8 = ml_dtypes.float8_e4m3

    # W8[p, kp, j, g, n] = WS * W_w.T[256*kp + 128*j + p, 512*gsrc + n]
    wt = (WS * W_w.T).reshape(KP, 2, 128, 4, 512)     # [kp, j, p, g', n]
    wt = wt.transpose(2, 0, 1, 3, 4)                  # [p, kp, j, g', n]
    wt = wt[:, :, :, G_SRC, :]
    w8 = np.ascontiguousarray(wt[:, 0:2]).astype(f8)
    wx = np.ascontiguousarray(wt[:, 2]).astype(bf)    # [p, j, g, n]

    br = np.zeros((1, 2, 4, 512), dtype=np.float32)
    br[0, 0] = (WS * W_b).reshape(4, 512)[list(G_SRC)]

    eye = np.eye(BC, dtype=np.float32)
    fcw = np.ascontiguousarray(np.broadcast_to(fc_w.reshape(1, H), (BC, H)))
    fcb = np.full((BC, 1), np.float32(fc_b[0]), dtype=np.float32)
    return {"W8": w8, "Wx": wx, "br8": br.astype(f8),
            "eye": eye.astype(bf), "eyef": eye, "fcw": fcw, "fcb": fcb}


def _prep_core_inputs(x, W_w, W_b, fc_w, fc_b, core, nsteps=T, shared=None):
    """Host-side shard + relayout for one core."""
    import ml_dtypes
    f8 = ml_dtypes.float8_e4m3
    if shared is None:
        shared = _prep_shared(W_w, W_b, fc_w, fc_b)

    import ml_dtypes as _md
    nxb = (nsteps + XB - 1) // XB
    xs = x[core * BC:(core + 1) * BC, :nsteps]          # [BC, t, I]
    xt = np.zeros((nxb * XB, 2, 128, BC), dtype=np.float32)
    xt[:nsteps] = xs.transpose(1, 2, 0).reshape(nsteps, 2, 128, BC)
    xt = xt.reshape(nxb, XB, 2, 128, BC).transpose(0, 3, 1, 2, 4)
    xb = np.ascontiguousarray(xt).astype(_md.bfloat16)  # [nxb,128,XB,2,BC]

    return {"xb": xb, **shared}


def kernel(x, W_w, W_b, fc_w, fc_b):
    x = np.asarray(x, dtype=np.float32)
    W_w = np.asarray(W_w, dtype=np.float32)
    W_b = np.asarray(W_b, dtype=np.float32)
    fc_w = np.asarray(fc_w, dtype=np.float32)
    fc_b = np.asarray(fc_b, dtype=np.float32)

    nc = _build(T)
    shared = _prep_shared(W_w, W_b, fc_w, fc_b)
    in_maps = [_prep_core_inputs(x, W_w, W_b, fc_w, fc_b, c, T, shared)
               for c in range(NCORES)]
    res = run_bass_kernel_spmd(nc, in_maps, list(range(NCORES))).results
    return np.concatenate([res[c]["out"] for c in range(NCORES)], axis=0)


# revision 19
# speedup vs baseline: 1.0001x; 1.0001x over previous
"""Trainium2 Bass kernel for CustomLSTMForecast.

B=512, T=256, I=256, H=512. Data-parallel: batch sharded 8 ways (64
rows/core), LSTM + fc weights replicated.

Per-core design (batch m = 64), fp8-DoubleRow recurrence:
  gates(t) = [h(t-1); x_t] @ (8*W_ext) + 8*bias   accumulated into two
  two-bank PSUM tiles (every matmul writes one bank at partition 0, as
  DoubleRow requires):
     PA[64,1024]: cols 0:512 = f-gate, 512:1024 = i-gate
     PB[64,1024]: cols 0:512 = c-hat,  512:1024 = o-gate
  The h-part runs in fp8e4 DoubleRow mode (K=256 per matmul, 2 k-pairs
  x 4 gates = 8 matmuls/step at half cost); h stored as fp8 loses only
  ~1e-2 final rel err (x-part in fp8 was 4e-2, so x stays bf16: 2
  k-chunks x 4 gates). Weights are pre-scaled by 8 on the host (keeps
  fp8 weights in a good range); the inverse 1/8 rides the ACT
  activations' free input scale. Bias is one Ki=1 fp8 DoubleRow matmul
  per gate (ones stationary, bias row moving).

  Elementwise (all partition-base 0): sigmoid(f,i) in one ACT op ->
  SBUF bf16; tanh(chat), sigmoid(o), tanh(c) -> SBUF bf16. The c/h tail
  is split into two hidden halves so each half's transposes, fp8 copy
  and next-step h-matmuls (A-bank gates first) overlap the other
  half's elementwise, shortening the recurrence critical path:
  u1 = c*sf, u2 = tcb*si, c = u1+u2 (all bf16, DVE 2x; bf16 c-state
  costs <1e-3 extra final error),
  h = so*tanh(c) (bf16), h -> 2 PE transposes -> one PSUM->SBUF copy
  = the fp8 [128,2,64] stationary for that k-pair of step t+1.

  x/bias matmuls for t+1 are emitted before the elementwise of t so
  the PE stays busy during the elementwise tail; x is DMA'd in blocks
  of 8 steps, triple buffered.
"""
from contextlib import ExitStack

import numpy as np

import concourse.bass as bass
import concourse.tile as tile
from concourse import bacc, mybir
from concourse.bass_utils import run_bass_kernel_spmd

F32 = mybir.dt.float32
BF16 = mybir.dt.bfloat16
FP8 = mybir.dt.float8e4
AF = mybir.ActivationFunctionType
DR = mybir.MatmulPerfMode.DoubleRow

B, T, I, H = 512, 256, 256, 512
NCORES = 8
BC = B // NCORES          # 64 batch rows per core
KP = 3                    # fp8 k-pairs: 2 hidden (512) + 1 input (256)
WS = 8.0                  # host-side weight scale, undone by ACT scale
XB = 8                    # x DMA batch (steps per transfer)

# gate order in the NG axis of the W tile: f, i, chat, o
# (W_w row-chunk order is f, i, o, chat)
G_SRC = (0, 1, 3, 2)

_CACHE = {}


def _build(nsteps=T):
    if nsteps in _CACHE:
        return _CACHE[nsteps]
    nc = bacc.Bacc("TRN2", target_bir_lowering=False, debug=False,
                   num_devices=NCORES)
    nxb = (nsteps + XB - 1) // XB
    d_x = nc.dram_tensor("xb", [nxb, 128, XB, 2, BC], BF16,
                         kind="ExternalInput").ap()
    d_w = nc.dram_tensor("W8", [128, 2, 2, 4, 512], FP8,
                         kind="ExternalInput").ap()
    d_wx = nc.dram_tensor("Wx", [128, 2, 4, 512], BF16,
                          kind="ExternalInput").ap()
    d_br = nc.dram_tensor("br8", [1, 2, 4, 512], FP8,
                          kind="ExternalInput").ap()
    d_eye = nc.dram_tensor("eye", [BC, BC], BF16, kind="ExternalInput").ap()
    d_eyef = nc.dram_tensor("eyef", [BC, BC], F32,
                            kind="ExternalInput").ap()
    d_fcw = nc.dram_tensor("fcw", [BC, H], F32, kind="ExternalInput").ap()
    d_fcb = nc.dram_tensor("fcb", [BC, 1], F32, kind="ExternalInput").ap()
    d_out = nc.dram_tensor("out", [BC, 1], F32, kind="ExternalOutput").ap()

    with tile.TileContext(nc) as tc, ExitStack() as ctx:
        _body(tc, ctx, nsteps, d_x, d_w, d_wx, d_br, d_eye, d_eyef,
              d_fcw, d_fcb, d_out)
    nc.compile()
    _CACHE[nsteps] = nc
    return nc


def _body(tc, ctx, nsteps, d_x, d_w, d_wx, d_br, d_eye, d_eyef, d_fcw,
          d_fcb, d_out):
    nc = tc.nc
    const = ctx.enter_context(tc.tile_pool(name="const", bufs=1))
    xpool = ctx.enter_context(tc.tile_pool(name="x", bufs=3))
    gact = ctx.enter_context(tc.tile_pool(name="gact", bufs=2))
    state = ctx.enter_context(tc.tile_pool(name="state", bufs=2))
    psA = ctx.enter_context(tc.tile_pool(name="psA", bufs=2, space="PSUM"))
    psB = ctx.enter_context(tc.tile_pool(name="psB", bufs=1, space="PSUM"))
    psT = ctx.enter_context(tc.tile_pool(name="psT", bufs=1, space="PSUM"))

    sW = const.tile([128, 2, 2, 4, 512], FP8)
    nc.sync.dma_start(out=sW[:], in_=d_w)
    sWx = const.tile([128, 2, 4, 512], BF16)
    nc.sync.dma_start(out=sWx[:], in_=d_wx)
    s_ones = const.tile([1, 2, BC], FP8)
    nc.vector.memset(s_ones[:, 0, :], 1.0)
    nc.vector.memset(s_ones[:, 1, :], 0.0)
    s_br = const.tile([1, 2, 4, 512], FP8)
    nc.sync.dma_start(out=s_br[:], in_=d_br)
    s_eye = const.tile([BC, BC], BF16)
    nc.sync.dma_start(out=s_eye[:], in_=d_eye)
    s_eyef = const.tile([BC, BC], F32)
    nc.sync.dma_start(out=s_eyef[:], in_=d_eyef)
    s_fcw = const.tile([BC, H], F32)
    nc.sync.dma_start(out=s_fcw[:], in_=d_fcw)
    s_fcb = const.tile([BC, 1], F32)
    nc.sync.dma_start(out=s_fcb[:], in_=d_fcb)

    c_prev = [state.tile([BC, 256], BF16, tag=f"c{i}", name=f"c{i}")
              for i in range(2)]
    nc.vector.memset(c_prev[0][:], 0.0)
    nc.vector.memset(c_prev[1][:], 0.0)

    xtiles = [None] * ((nsteps + XB - 1) // XB)

    def get_x(t):
        blk = t // XB
        if xtiles[blk] is None:
            xt = xpool.tile([128, XB, 2, BC], BF16, tag="xs")
            nc.sync.dma_start(out=xt[:], in_=d_x[blk])
            xtiles[blk] = xt
        return xtiles[blk][:, t % XB]

    def bank(pA, pB, g):
        ps = pA if g < 2 else pB
        return ps[:, 512 * (g % 2):512 * (g % 2) + 512]

    def emit_x_and_bias(t, pA, pB):
        """x-part (bf16) + bias (fp8 Ki=1 DoubleRow) matmuls for step t."""
        xs = get_x(t)                       # [128, 2, BC] bf16
        for g in range(4):
            out = bank(pA, pB, g)
            for j in range(2):
                nc.tensor.matmul(out, xs[:, j, :], sWx[:, j, g, :],
                                 start=(j == 0), stop=False)
            nc.tensor.matmul(out, s_ones[:], s_br[:, :, g, :],
                             start=False, stop=(t == 0), perf_mode=DR)

    def emit_hpart(pA, pB, hq, kp):
        """h-part fp8 DoubleRow matmuls for one k-pair (A gates first)."""
        for g in range(4):
            nc.tensor.matmul(bank(pA, pB, g), hq[:],
                             sW[:, kp, :, g, :],
                             start=False, stop=(kp == 1), perf_mode=DR)

    # prologue: step 0 gates have no h contribution
    pA = psA.tile([BC, 1024], F32, tag="A")
    pB = psB.tile([BC, 1024], F32, tag="B")
    emit_x_and_bias(0, pA, pB)

    h_half = None
    for t in range(nsteps):
        last = t == nsteps - 1
        if not last:
            pA_n = psA.tile([BC, 1024], F32, tag="A")
            pB_n = psB.tile([BC, 1024], F32, tag="B")
            emit_x_and_bias(t + 1, pA_n, pB_n)

        # gate activations for step t (PA = [f|i], PB = [chat|o])
        sfi = gact.tile([BC, 1024], BF16, tag="sfi")
        nc.scalar.activation(sfi[:], pA[:], AF.Sigmoid, scale=1.0 / WS)
        tcb = gact.tile([BC, 512], BF16, tag="tcb")
        nc.scalar.activation(tcb[:], pB[:, 0:512], AF.Tanh, scale=1.0 / WS)
        sigo = gact.tile([BC, 512], BF16, tag="sigo")
        nc.scalar.activation(sigo[:], pB[:, 512:1024], AF.Sigmoid,
                             scale=1.0 / WS)
        # c update, split into two hidden halves
        c_new = [state.tile([BC, 256], BF16, tag=f"c{i}", name=f"c{i}")
                 for i in range(2)]
        u1 = [gact.tile([BC, 256], BF16, tag=f"u1{i}", name=f"u1{i}")
              for i in range(2)]
        u2 = [gact.tile([BC, 256], BF16, tag=f"u2{i}", name=f"u2{i}")
              for i in range(2)]
        for i in range(2):
            s = 256 * i
            nc.vector.tensor_mul(u1[i][:], c_prev[i][:], sfi[:, s:s + 256])
            nc.vector.tensor_mul(u2[i][:], tcb[:, s:s + 256],
                                 sfi[:, 512 + s:512 + s + 256])
        for i in range(2):
            nc.vector.tensor_add(c_new[i][:], u1[i][:], u2[i][:])
        c_prev = c_new

        h_half = []
        for i in range(2):
            s = 256 * i
            tch = gact.tile([BC, 256], BF16, tag=f"tch{i}", name=f"tch{i}")
            nc.scalar.activation(tch[:], c_new[i][:], AF.Tanh)
            hh = state.tile([BC, 256], BF16, tag=f"h{i}", name=f"h{i}")
            nc.vector.tensor_mul(hh[:], sigo[:, s:s + 256], tch[:])
            h_half.append(hh)
            if not last:
                pT = psT.tile([128, 2 * BC], BF16, tag=f"hTp{i}",
                              name=f"hTp{i}")
                for j in range(2):
                    nc.tensor.transpose(pT[:, 64 * j:64 * j + 64],
                                        hh[:, 128 * j:128 * j + 128],
                                        s_eye[:])
                hq = state.tile([128, 2, BC], FP8, tag=f"hq{i}",
                                name=f"hq{i}")
                nc.vector.tensor_copy(hq[:], pT[:])
                emit_hpart(pA_n, pB_n, hq, i)
        if not last:
            pA, pB = pA_n, pB_n

    # fc head: out = h @ fc_w.T + fc_b
    m = gact.tile([BC, H], F32, tag="fcm")
    for i in range(2):
        nc.vector.tensor_mul(m[:, 256 * i:256 * i + 256], h_half[i][:],
                             s_fcw[:, 256 * i:256 * i + 256])
    r = gact.tile([BC, 1], F32, tag="fcr")
    nc.vector.tensor_reduce(r[:], m[:], axis=mybir.AxisListType.X,
                            op=mybir.AluOpType.add)
    ro = gact.tile([BC, 1], F32, tag="fco")
    nc.vector.tensor_add(ro[:], r[:], s_fcb[:])
    nc.sync.dma_start(out=d_out, in_=ro[:])


def _prep_shared(W_w, W_b, fc_w, fc_b):
    """Host-side weight relayout (core-independent)."""
    import ml_dtypes
    bf = ml_dtypes.bfloat16
    f8 = ml_dtypes.float8_e4m3

    # W8[p, kp, j, g, n] = WS * W_w.T[256*kp + 128*j + p, 512*gsrc + n]
    wt = (WS * W_w.T).reshape(KP, 2, 128, 4, 512)     # [kp, j, p, g', n]
    wt = wt.transpose(2, 0, 1, 3, 4)                  # [p, kp, j, g', n]
    wt = wt[:, :, :, G_SRC, :]
    w8 = np.ascontiguousarray(wt[:, 0:2]).astype(f8)
    wx = np.ascontiguousarray(wt[:, 2]).astype(bf)    # [p, j, g, n]

    br = np.zeros((1, 2, 4, 512), dtype=np.float32)
    br[0, 0] = (WS * W_b).reshape(4, 512)[list(G_SRC)]

    eye = np.eye(BC, dtype=np.float32)
    fcw = np.ascontiguousarray(np.broadcast_to(fc_w.reshape(1, H), (BC, H)))
    fcb = np.full((BC, 1), np.float32(fc_b[0]), dtype=np.float32)
    return {"W8": w8, "Wx": wx, "br8": br.astype(f8),
            "eye": eye.astype(bf), "eyef": eye, "fcw": fcw, "fcb": fcb}


def _prep_core_inputs(x, W_w, W_b, fc_w, fc_b, core, nsteps=T, shared=None):
    """Host-side shard + relayout for one core."""
    import ml_dtypes
    f8 = ml_dtypes.float8_e4m3
    if shared is None:
        shared = _prep_shared(W_w, W_b, fc_w, fc_b)

    import ml_dtypes as _md
    nxb = (nsteps + XB - 1) // XB
    xs = x[core * BC:(core + 1) * BC, :nsteps]          # [BC, t, I]
    xt = np.zeros((nxb * XB, 2, 128, BC), dtype=np.float32)
    xt[:nsteps] = xs.transpose(1, 2, 0).reshape(nsteps, 2, 128, BC)
    xt = xt.reshape(nxb, XB, 2, 128, BC).transpose(0, 3, 1, 2, 4)
    xb = np.ascontiguousarray(xt).astype(_md.bfloat16)  # [nxb,128,XB,2,BC]

    return {"xb": xb, **shared}


def kernel(x, W_w, W_b, fc_w, fc_b):
    x = np.asarray(x, dtype=np.float32)
    W_w = np.asarray(W_w, dtype=np.float32)
    W_b = np.asarray(W_b, dtype=np.float32)
    fc_w = np.asarray(fc_w, dtype=np.float32)
    fc_b = np.asarray(fc_b, dtype=np.float32)

    nc = _build(T)
    shared = _prep_shared(W_w, W_b, fc_w, fc_b)
    in_maps = [_prep_core_inputs(x, W_w, W_b, fc_w, fc_b, c, T, shared)
               for c in range(NCORES)]
    res = run_bass_kernel_spmd(nc, in_maps, list(range(NCORES))).results
    return np.concatenate([res[c]["out"] for c in range(NCORES)], axis=0)
